# revision 36
# baseline (speedup 1.0000x reference)
"""Device kernels + host middle for nn_Entropy_Hist (3x3x3 window entropy
histogram + top-k channel gather) on 8 trn2 cores.

Phase 1 (device): per core 16 channel slabs (8 pairs, partition = h of 2
slabs). Per pair: ONE contiguous DMA load [128, 64*64], a single fp8
cast pass of x on DVE, then per w-chunk FIVE matmuls: four fp8
DoubleRow band matmuls whose k-tile pairs cover the 8 non-middle (w,z)
tap columns (each tap column is a 3-h band sum), plus one f32r bandc
matmul for the middle column (w+1,z+1) carrying its 2 h-neighbors at
f32 precision and the center term on the diagonal. Then a FUSED
evac+quantize: one activation op PSUM -> u16 with a FIXED compile-time
scale (no per-pair absmax / reciprocal on the critical path):
    q16 = round_half_even( ij * (32767/B_FIX) + 32767 ),  saturating.
B_FIX bounds |ij|; saturation shows up as q16 in {0, 65535} which the
host detects and repairs by exact recompute of the affected rows.

Host middle: decode ij from q16 with the fixed scale, locate exact
global min/max among decoded-extreme candidates (recomputed exactly),
bin all samples, recompute near-boundary (flagged) samples exactly,
entropy + top-k as reference.

Phase 2 (device): gather selected channel rows, column-sharded across
cores, consecutive selected rows batched into single DMAs (device emits
sorted row order; host restores top-k order).
"""

import ml_dtypes
import numpy as np

import concourse.bass as bass
import concourse.bacc as bacc
import concourse.mybir as mybir
import concourse.tile as tile
from concourse.bass_utils import run_bass_kernel_spmd

N_CORES = 8
B, C, H, W, Z = 2, 64, 64, 64, 64
HP = H - 2              # 62 valid per spatial dim
FD = HP * HP            # 3844 free elems per partition (w', z')
P_SLAB = HP * HP * HP   # 238328 voxels per slab
SLABS_PER_CORE = (B * C) // N_CORES  # 16
PAIRS = SLABS_PER_CORE // 2          # 8
K26 = np.float32(1.0) / np.float32(26.0)
CDIAG = np.float32(100.0) - K26      # center coefficient
BINS = 256
DENOM = (H + 2) * (W + 2) * (Z + 2)

# fixed 12-bit quantization grid: q12 = round(ij * QS + QBIA) in [0, 4080].
# Each adjacent sample pair is packed as v24 = q_even + 4096*q_odd and
# emitted as u8 hi = round(v24/2^16) (exact: ties impossible for q>=2)
# plus i16 lo = v24 - hi*2^16  -> 1.5 bytes/sample.
B_FIX = 640.0            # bound on |ij| (dataset max ~542); q12 near 0 or
QBIA = 2040.0            # 4080 is detected on host and repaired exactly
QS = 2040.0 / B_FIX
ULP = B_FIX / 2040.0     # decode step (7.5% of a bin)
FLAG_T = 0.088           # bin-fraction margin -> host recomputes exactly
                         # (0.5 ulp = 3.8% + device arith 3.6% + safety)
FDH = FD // 2            # 1922 packed sample-pairs per partition

# fp8 weight grid: the X2 (non-center) taps run through fp8 DoubleRow
# matmuls with weight BETA; the evac rescales by K26/BETA
BETA = np.float32(0.0390625)          # 1/25.6, exact in e4m3
SCORR = np.float64(K26) / np.float64(BETA)          # evac scale
CDIAG_ADJ = np.float32(np.float64(CDIAG) / SCORR)   # center diag pre-descale
EVAC_SCALE = float(np.float64(SCORR) * np.float64(QS))  # psum -> q units


def build_band(w=BETA):
    """[128,128] f32: col m sums rows m-1..m+1 (within each 64 block) with
    weight w. Cols 0,63,64,127 are all-zero, so the garbage partitions
    hold exact 0 (which quantizes to QBIA: harmless, never read)."""
    band = np.zeros((128, 128), np.float32)
    for blk in (0, 64):
        for m in range(1, 63):
            for k in (m - 1, m, m + 1):
                band[blk + k, blk + m] = w
    return band


def build_bandc():
    """beta-band + CDIAG_ADJ * I on valid cols: the middle tap column
    (w+1, z+1) in f32r — its two h-neighbors at weight BETA plus the
    center term on the diagonal (evac scale SCORR restores CDIAG)."""
    band = build_band()
    for blk in (0, 64):
        for m in range(1, 63):
            band[blk + m, blk + m] += CDIAG_ADJ
    return band


def build_band8():
    """fp8 DoubleRow stationary [128, 2*128] (k-tile major):
    S1 = [band | band] — both k-tiles contract a tap column with the
    3-h band at weight BETA."""
    b8 = build_band().astype(ml_dtypes.float8_e4m3)
    s1 = np.concatenate([b8, b8], axis=1)
    return s1


# the 8 non-middle (w,z) tap columns, paired into 4 DoubleRow matmuls:
# each entry is (dw, dz, ktile_stride_elems) with the second k-tile at
# flat offset +stride in the [p, w, z] fp8 tile (z stride 1, w stride Z)
DR_TAPS = [
    (0, 0, 1),       # (0,0) + (0,1)
    (0, 2, Z - 2),   # (0,2) + (1,0)
    (1, 2, Z - 2),   # (1,2) + (2,0)
    (2, 1, 1),       # (2,1) + (2,2)
]


def _dr_rhs(xf3, w0, wn, dw, dz, stride):
    """Moving AP [128, 2(k-tile), wn, 62]: k-tile 0 at (w0+dw, dz), k-tile
    1 at flat offset +stride (overlapping strided dims)."""
    v = xf3[:, w0 + dw:w0 + dw + wn, dz:dz + HP]
    ap = [list(v.ap[0]), [stride, 2], list(v.ap[1]), list(v.ap[2])]
    return type(v)(v.tensor, v.offset, ap)


def build_phase1():
    nc = bacc.Bacc("TRN2", target_bir_lowering=False, debug=False,
                   num_devices=N_CORES)
    f32, f32r = mybir.dt.float32, mybir.dt.float32r
    u16 = mybir.dt.uint16
    f8 = mybir.dt.float8e4
    imgp = nc.dram_tensor("imgp", [SLABS_PER_CORE, H, W, Z], f32r,
                          kind="ExternalInput")
    bandcw = nc.dram_tensor("bandcw", [128, 128], f32r, kind="ExternalInput")
    s1w = nc.dram_tensor("s1w", [128, 256], f8, kind="ExternalInput")
    # pairs 0..6 ship packed 12-bit (hi u8 + lo i16); the last pair ships
    # raw u16 q12 so the epilogue has no pack chain
    hi_o = nc.dram_tensor("hi", [SLABS_PER_CORE - 2, P_SLAB // 2],
                          mybir.dt.uint8, kind="ExternalOutput")
    lo_o = nc.dram_tensor("lo", [SLABS_PER_CORE - 2, P_SLAB // 2],
                          mybir.dt.int16, kind="ExternalOutput")
    qlast_o = nc.dram_tensor("qlast", [2, P_SLAB], u16,
                             kind="ExternalOutput")

    # w' chunking for PSUM banks: chunks of 8 w' rows (<=496 free each)
    W_CHUNKS = [(i, min(8, HP - i)) for i in range(0, HP, 8)]

    with tile.TileContext(nc) as tc:
        with (
            tc.tile_pool(name="pool", bufs=1) as pool,
            tc.tile_pool(name="pimg", bufs=7) as pimg,
            tc.tile_pool(name="px8", bufs=2) as px8,
            tc.tile_pool(name="pq", bufs=2) as pq,
            tc.tile_pool(name="pv", bufs=4) as pv,
            tc.tile_pool(name="phl", bufs=3) as phl,
            tc.tile_pool(name="psum", bufs=8, space="PSUM") as psum,
        ):
            bandc_t = pool.tile([128, 128], f32r, tag="bandc")
            s1_t = pool.tile([128, 256], f8, tag="s1")
            s1_3 = s1_t[:].rearrange("p (t m) -> p t m", t=2)

            qbia_t = pool.tile([128, 1], mybir.dt.float32, tag="qbia")
            nc.vector.memset(qbia_t[:], QBIA)

            img_tiles = [None] * PAIRS
            x8_tiles = [None] * PAIRS
            q12_tiles = [None] * PAIRS
            v24_tiles = [None] * PAIRS
            hi_tiles = [None] * PAIRS
            lo_tiles = [None] * PAIRS

            def emit_load(p):
                src = imgp[2 * p:2 * p + 2].rearrange("s h w z -> (s h) (w z)")
                t = pimg.tile([128, W * Z], f32r, tag="img")
                img_tiles[p] = t
                nc.sync.dma_start(t[:], src)

            CSPL = 48 * Z  # cast split: w 0:48 on DVE, 48:64 on Pool

            def emit_cast_d(p):
                x8 = px8.tile([128, W * Z], f8, tag="x8")
                x8_tiles[p] = x8
                nc.vector.tensor_scalar(x8[:, 0:CSPL],
                                        img_tiles[p][:, 0:CSPL], 1.0, None,
                                        mybir.AluOpType.mult)

            def emit_cast_p(p):
                nc.gpsimd.tensor_scalar(x8_tiles[p][:, CSPL:],
                                        img_tiles[p][:, CSPL:], 1.0, None,
                                        mybir.AluOpType.mult)

            def emit_pack_v(p):
                # v24 halves on DVE: q12(p) is fully written one pair ago,
                # so these run dependency-free at iteration start
                q3 = q12_tiles[p][:].rearrange("p (f two) -> p two f", two=2)
                va = pv.tile([128, FDH // 2], mybir.dt.float32, tag="va")
                vb = pv.tile([128, FDH // 2], mybir.dt.float32, tag="vb")
                v24_tiles[p] = (va, vb)
                hit = phl.tile([128, FDH], mybir.dt.uint8, tag="hi")
                lot = phl.tile([128, FDH], mybir.dt.int16, tag="lo")
                hi_tiles[p] = hit
                lo_tiles[p] = lot
                for h, v in ((0, va), (1, vb)):
                    sl = slice(h * (FDH // 2), (h + 1) * (FDH // 2))
                    nc.vector.scalar_tensor_tensor(
                        v[:], q3[:, 1, sl], 4096.0, q3[:, 0, sl],
                        mybir.AluOpType.mult, mybir.AluOpType.add)

            def emit_pack_hi(p):
                # hi = round(v24 / 2^16) as u8: half a on Act (rides ahead
                # of the evac stream), half b on Pool
                va, vb = v24_tiles[p]
                nc.scalar.activation(hi_tiles[p][:, 0:FDH // 2], va[:],
                                     mybir.ActivationFunctionType.Copy,
                                     scale=float(2.0 ** -16))
                nc.gpsimd.tensor_scalar(hi_tiles[p][:, FDH // 2:], vb[:],
                                        float(2.0 ** -16), None,
                                        mybir.AluOpType.mult)

            def emit_pack_lo(p):
                # lo = v24 - 65536*hi as i16 on DVE (exact)
                for h, v in ((0, v24_tiles[p][0]), (1, v24_tiles[p][1])):
                    sl = slice(h * (FDH // 2), (h + 1) * (FDH // 2))
                    nc.vector.scalar_tensor_tensor(
                        lo_tiles[p][:, sl], hi_tiles[p][:, sl], -65536.0,
                        v[:], mybir.AluOpType.mult, mybir.AluOpType.add)

            def emit_outs(p):
                # 4 writes on the SP queue (loads are long since issued)
                for half in range(2):
                    s = 2 * p + half
                    rows = slice(64 * half + 1, 64 * half + 63)
                    nc.sync.dma_start(
                        hi_o[s].rearrange("(h f) -> h f", h=HP),
                        hi_tiles[p][rows, :])
                    nc.sync.dma_start(
                        lo_o[s].rearrange("(h f) -> h f", h=HP),
                        lo_tiles[p][rows, :])

            # prologue: first data load ahead of the weight DMAs, then the
            # rest of the loads stream back-to-back on the SP queue
            emit_load(0)
            nc.sync.dma_start(bandc_t[:], bandcw[:])
            nc.sync.dma_start(s1_t[:], s1w[:])
            for p in range(1, PAIRS):
                emit_load(p)
            emit_cast_d(0)
            emit_cast_p(0)

            for p in range(PAIRS):
                if p >= 1:
                    emit_pack_v(p - 1)        # DVE: dependency-free packs
                if p + 1 < PAIRS:
                    emit_cast_d(p + 1)
                    emit_cast_p(p + 1)
                if p >= 1:
                    emit_pack_hi(p - 1)       # Pool, after v24 halves

                xv = img_tiles[p][:].rearrange("p (w z) -> p w z", w=W)
                xf3 = x8_tiles[p][:].rearrange("p (w z) -> p w z", w=W)
                q12 = pq.tile([128, FD], u16, tag="q12")
                q12_tiles[p] = q12
                for ci, (w0, wn) in enumerate(W_CHUNKS):
                    ps = psum.tile([128, 8 * HP], f32, tag="ps")
                    out_ap = ps[:, 0:wn * HP]
                    # 4 fp8 DoubleRow matmuls: 8 non-middle tap columns
                    for ti, (dw, dz, st) in enumerate(DR_TAPS):
                        nc.tensor.matmul(
                            out_ap, s1_3, _dr_rhs(xf3, w0, wn, dw, dz, st),
                            start=(ti == 0), stop=False,
                            perf_mode=mybir.MatmulPerfMode.DoubleRow)
                    # middle column (w+1, z+1): band + center diag in f32r
                    nc.tensor.matmul(out_ap, bandc_t[:],
                                     xv[:, w0 + 1:w0 + 1 + wn, 1:1 + HP],
                                     start=False, stop=True)
                    # fused evac + fixed-scale quantize (saturating u16)
                    sl = slice(w0 * HP, (w0 + wn) * HP)
                    nc.scalar.activation(
                        q12[:, sl], out_ap,
                        mybir.ActivationFunctionType.Identity,
                        scale=EVAC_SCALE, bias=qbia_t[:])
                if p >= 1:
                    emit_pack_lo(p - 1)       # DVE, after Pool's hi
                    emit_outs(p - 1)

            # epilogue: last pair ships raw u16 q12 (no pack chain)
            for half in range(2):
                rows = slice(64 * half + 1, 64 * half + 63)
                nc.sync.dma_start(
                    qlast_o[half].rearrange("(h f) -> h f", h=HP),
                    q12_tiles[PAIRS - 1][rows, :])

    nc.finalize()
    return nc


def _stride_runs(rows):
    """Group a sorted int list into (start, stride, count) constant-stride
    runs (each run becomes one strided DMA access pattern)."""
    runs = []
    i, n = 0, len(rows)
    while i < n:
        if i + 1 == n:
            runs.append((rows[i], 1, 1))
            break
        d = rows[i + 1] - rows[i]
        j = i + 1
        while j + 1 < n and rows[j + 1] - rows[j] == d:
            j += 1
        runs.append((rows[i], d, j - i + 1))
        i = j + 1
    return runs


def build_phase2(sel_rows_sorted):
    """sel_rows_sorted: ascending flat row ids (b*C+c); identical program on
    all cores; each core handles one column-chunk of every selected row.
    Consecutive rows are batched into single DMAs."""
    n_sel = len(sel_rows_sorted)
    CHUNK = (H * W * Z) // N_CORES
    nc = bacc.Bacc("TRN2", target_bir_lowering=False, debug=False,
                   num_devices=N_CORES)
    f32 = mybir.dt.float32
    img = nc.dram_tensor("imgchunk", [B * C, CHUNK], f32,
                         kind="ExternalInput")
    out = nc.dram_tensor("sel", [n_sel, CHUNK], f32, kind="ExternalOutput")
    with tile.TileContext(nc):
        j = 0
        engines = [nc.sync, nc.scalar, nc.vector, nc.gpsimd]
        for i, (r0, d, cnt) in enumerate(
                _stride_runs([int(r) for r in sel_rows_sorted])):
            engines[i % len(engines)].dma_start(
                out[j:j + cnt, :], img[r0:r0 + (cnt - 1) * d + 1:d, :])
            j += cnt
    nc.finalize()
    return nc, n_sel


# ---------------------------------------------------------------------------
# host middle
# ---------------------------------------------------------------------------


def unpack_q12(hi, lo, qlast):
    """hi: [n_cores, 14, P_SLAB//2] u8, lo: same-shape i16, qlast:
    [n_cores, 2, P_SLAB] u16 -> q12 [B*C, P_SLAB] int64."""
    n_cores = hi.shape[0]
    v24 = hi.astype(np.int64) * 65536 + lo.astype(np.int64)
    q_odd = v24 >> 12
    q_even = v24 - (q_odd << 12)
    q12 = np.empty((n_cores, SLABS_PER_CORE, HP, FD), np.int64)
    q12[:, :14, :, 0::2] = q_even.reshape(n_cores, 14, HP, FD // 2)
    q12[:, :14, :, 1::2] = q_odd.reshape(n_cores, 14, HP, FD // 2)
    q12[:, 14:] = qlast.astype(np.int64).reshape(n_cores, 2, HP, FD)
    return q12.reshape(B * C, P_SLAB)


def host_middle(img, k, q12, jnp, jax):
    """q12: [B*C, P_SLAB] int64 fixed-grid samples in device (h',w',z')
    order. Returns idx [B, k]."""
    nrows = B * C
    imgf = np.asarray(img)

    def exact_ij(rs, fs):
        hq, rem = np.divmod(fs, HP * HP)
        wq, zq = np.divmod(rem, HP)
        bq, cq = np.divmod(rs, C)
        s = np.zeros(len(rs), np.float32)
        for di in range(3):
            for dj in range(3):
                for dk in range(3):
                    s = s + imgf[bq, cq, hq + di, wq + dj, zq + dk]
        cen = imgf[bq, cq, hq + 1, wq + 1, zq + 1]
        mean_p = (s - cen) / np.float32(26.0)
        return cen * np.float32(100.0) + mean_p

    # saturation / pack-ambiguity repair: q12 near 0 or 4080 means |ij|
    # may have exceeded B_FIX (or hit the hi-rounding tie) on that row ->
    # recompute the whole row exactly
    sat_rows = np.unique(np.nonzero((q12 <= 1) | (q12 >= 4079))[0])
    ij_dec = (q12.astype(np.float64) - QBIA) * ULP
    for r in sat_rows:
        fs = np.arange(P_SLAB)
        ij_dec[r] = exact_ij(np.full(P_SLAB, r), fs).astype(np.float64)

    # exact global min/max: candidates = decoded values near the decoded
    # extremes (true extreme is within decode ulp + device-arith error
    # of the decoded one; 0.1 ij-units covers the arithmetic tail)
    mn_d = ij_dec.min()
    mx_d = ij_dec.max()
    win = 2.5 * ULP + 0.5
    cand = (ij_dec <= mn_d + win) | (ij_dec >= mx_d - win)
    crs, cfs = np.nonzero(cand)
    cij = exact_ij(crs, cfs)
    mn = np.float32(cij.min())
    mx = np.float32(cij.max())

    # provisional bins + boundary flags from decoded values
    qc = (ij_dec - mn) * (np.float64(BINS) / np.float64(mx - mn))
    binf = np.floor(qc)
    frac = qc - binf
    bins = np.clip(binf, 0, BINS - 1).astype(np.int64)
    flag = (frac < FLAG_T) | (frac > 1.0 - FLAG_T) | (binf < 0) | \
           (binf > BINS - 1)
    del qc, binf, frac, ij_dec

    hist = np.zeros((nrows, BINS), np.int64)
    for r in range(nrows):
        hist[r] = np.bincount(bins[r], minlength=BINS)

    # flagged: recompute exactly in reference f32 arithmetic and move count
    rs, fs = np.nonzero(flag)
    ij_ref = exact_ij(rs, fs)
    q = (ij_ref - mn) / (mx - mn)
    true_bin = np.clip(np.floor(q * np.float32(BINS)), 0, BINS - 1).astype(np.int64)
    dev_bin = bins[rs, fs]
    np.subtract.at(hist, (rs, dev_bin), 1)
    np.add.at(hist, (rs, true_bin), 1)

    # entropy + topk exactly as reference (jax CPU)
    cpu = jax.devices("cpu")[0]
    with jax.default_device(cpu):
        h = jnp.asarray(hist.astype(np.float32))
        p = h / DENOM
        h_tem = -p * jnp.log(jnp.clip(p, 1e-40)) / np.float32(np.log(2.0))
        ent = h_tem.sum(axis=1).reshape(B, C)
        _, idx = jax.lax.top_k(ent, int(k))
        idx = np.asarray(idx)
    return idx, hist, (mn, mx)


LAST_NCS = [None, None]  # (nc1, nc2) from the most recent run_full


def run_full(img, k, trace=False):
    import jax
    import jax.numpy as jnp
    img = np.asarray(img, dtype=np.float32)
    k = int(k)

    nc1 = build_phase1()
    bandc = build_bandc()
    s1 = build_band8()
    imgr = img.reshape(B * C, H, W, Z)
    in_maps = [{"imgp": np.ascontiguousarray(imgr[16 * c:16 * c + 16]),
                "bandcw": bandc, "s1w": s1}
               for c in range(N_CORES)]
    res1 = run_bass_kernel_spmd(nc1, in_maps, core_ids=list(range(N_CORES)),
                                trace=trace)
    hi = np.stack([res1.results[c]["hi"] for c in range(N_CORES)], 0)
    lo = np.stack([res1.results[c]["lo"] for c in range(N_CORES)], 0)
    qlast = np.stack([res1.results[c]["qlast"] for c in range(N_CORES)], 0)
    q12 = unpack_q12(hi, lo, qlast)

    idx, hist, mnmx = host_middle(img, k, q12, jnp, jax)

    # phase 2: device gather of selected slabs, column-sharded over cores;
    # device writes sorted row order, host restores top-k order
    rows_flat = np.array([int(b * C + ch) for b in range(B) for ch in idx[b]])
    order = np.argsort(rows_flat, kind="stable")
    rows_sorted = rows_flat[order]
    inv = np.empty_like(order)
    inv[order] = np.arange(len(order))

    nc2, n_sel = build_phase2(rows_sorted.tolist())
    LAST_NCS[0], LAST_NCS[1] = nc1, nc2
    CHUNK = (H * W * Z) // N_CORES
    img2 = img.reshape(B * C, H * W * Z)
    in2 = [{"imgchunk": np.ascontiguousarray(img2[:, c * CHUNK:(c + 1) * CHUNK])}
           for c in range(N_CORES)]
    res2 = run_bass_kernel_spmd(nc2, in2, core_ids=list(range(N_CORES)),
                                trace=trace)

    out_sorted = np.zeros((n_sel, H * W * Z), np.float32)
    for c in range(N_CORES):
        out_sorted[:, c * CHUNK:(c + 1) * CHUNK] = res2.results[c]["sel"]
    out = out_sorted[inv].reshape(B, k, H, W, Z)
    return out, (res1, res2)


def kernel(**inputs):
    """Entry point: full inputs in, full output out."""
    img = np.asarray(inputs["img"], dtype=np.float32)
    k = int(np.asarray(inputs["k"]))
    out, _ = run_full(img, k)
    return out.astype(np.float32)


# revision 38
# speedup vs baseline: 1.0639x; 1.0639x over previous
"""Device kernels + host middle for nn_Entropy_Hist (3x3x3 window entropy
histogram + top-k channel gather) on 8 trn2 cores.

Phase 1 (device): per core 16 channel slabs (8 pairs, partition = h of 2
slabs). Per pair: ONE contiguous DMA load [128, 64*64], a single fp8
cast pass of x on DVE, then per w-chunk FIVE matmuls: four fp8
DoubleRow band matmuls whose k-tile pairs cover the 8 non-middle (w,z)
tap columns (each tap column is a 3-h band sum), plus one f32r bandc
matmul for the middle column (w+1,z+1) carrying its 2 h-neighbors at
f32 precision and the center term on the diagonal. Then a FUSED
evac+quantize: one activation op PSUM -> u16 with a FIXED compile-time
scale (no per-pair absmax / reciprocal on the critical path):
    q16 = round_half_even( ij * (32767/B_FIX) + 32767 ),  saturating.
B_FIX bounds |ij|; saturation shows up as q16 in {0, 65535} which the
host detects and repairs by exact recompute of the affected rows.

Host middle: decode ij from q16 with the fixed scale, locate exact
global min/max among decoded-extreme candidates (recomputed exactly),
bin all samples, recompute near-boundary (flagged) samples exactly,
entropy + top-k as reference.

Phase 2 (device): gather selected channel rows, column-sharded across
cores, consecutive selected rows batched into single DMAs (device emits
sorted row order; host restores top-k order).
"""

import ml_dtypes
import numpy as np

import concourse.bass as bass
import concourse.bacc as bacc
import concourse.mybir as mybir
import concourse.tile as tile
from concourse.bass_utils import run_bass_kernel_spmd

N_CORES = 8
B, C, H, W, Z = 2, 64, 64, 64, 64
HP = H - 2              # 62 valid per spatial dim
FD = HP * HP            # 3844 free elems per partition (w', z')
P_SLAB = HP * HP * HP   # 238328 voxels per slab
SLABS_PER_CORE = (B * C) // N_CORES  # 16
PAIRS = SLABS_PER_CORE // 2          # 8
K26 = np.float32(1.0) / np.float32(26.0)
CDIAG = np.float32(100.0) - K26      # center coefficient
BINS = 256
DENOM = (H + 2) * (W + 2) * (Z + 2)

# fixed 12-bit quantization grid: q12 = round(ij * QS + QBIA) in [0, 4080].
# Each adjacent sample pair is packed as v24 = q_even + 4096*q_odd and
# emitted as u8 hi = round(v24/2^16) (exact: ties impossible for q>=2)
# plus i16 lo = v24 - hi*2^16  -> 1.5 bytes/sample.
B_FIX = 640.0            # bound on |ij| (dataset max ~542); q12 near 0 or
QBIA = 2040.0            # 4080 is detected on host and repaired exactly
QS = 2040.0 / B_FIX
ULP = B_FIX / 2040.0     # decode step (7.5% of a bin)
FLAG_T = 0.088           # bin-fraction margin -> host recomputes exactly
                         # (0.5 ulp = 3.8% + device arith 3.6% + safety)
FDH = FD // 2            # 1922 packed sample-pairs per partition

# fp8 weight grid: the X2 (non-center) taps run through fp8 DoubleRow
# matmuls with weight BETA; the evac rescales by K26/BETA
BETA = np.float32(0.0390625)          # 1/25.6, exact in e4m3
SCORR = np.float64(K26) / np.float64(BETA)          # evac scale
CDIAG_ADJ = np.float32(np.float64(CDIAG) / SCORR)   # center diag pre-descale
EVAC_SCALE = float(np.float64(SCORR) * np.float64(QS))  # psum -> q units


def build_band(w=BETA):
    """[128,128] f32: col m sums rows m-1..m+1 (within each 64 block) with
    weight w. Cols 0,63,64,127 are all-zero, so the garbage partitions
    hold exact 0 (which quantizes to QBIA: harmless, never read)."""
    band = np.zeros((128, 128), np.float32)
    for blk in (0, 64):
        for m in range(1, 63):
            for k in (m - 1, m, m + 1):
                band[blk + k, blk + m] = w
    return band


def build_bandc():
    """beta-band + CDIAG_ADJ * I on valid cols: the middle tap column
    (w+1, z+1) in f32r — its two h-neighbors at weight BETA plus the
    center term on the diagonal (evac scale SCORR restores CDIAG)."""
    band = build_band()
    for blk in (0, 64):
        for m in range(1, 63):
            band[blk + m, blk + m] += CDIAG_ADJ
    return band


def build_band8():
    """fp8 DoubleRow stationary [128, 2*128] (k-tile major):
    S1 = [band | band] — both k-tiles contract a tap column with the
    3-h band at weight BETA."""
    b8 = build_band().astype(ml_dtypes.float8_e4m3)
    s1 = np.concatenate([b8, b8], axis=1)
    return s1


# the 8 non-middle (w,z) tap columns, paired into 4 DoubleRow matmuls:
# each entry is (dw, dz, ktile_stride_elems) with the second k-tile at
# flat offset +stride in the [p, w, z] fp8 tile (z stride 1, w stride Z)
DR_TAPS = [
    (0, 0, 1),       # (0,0) + (0,1)
    (0, 2, Z - 2),   # (0,2) + (1,0)
    (1, 2, Z - 2),   # (1,2) + (2,0)
    (2, 1, 1),       # (2,1) + (2,2)
]


def _dr_rhs(xf3, w0, wn, dw, dz, stride):
    """Moving AP [128, 2(k-tile), wn, 62]: k-tile 0 at (w0+dw, dz), k-tile
    1 at flat offset +stride (overlapping strided dims)."""
    v = xf3[:, w0 + dw:w0 + dw + wn, dz:dz + HP]
    ap = [list(v.ap[0]), [stride, 2], list(v.ap[1]), list(v.ap[2])]
    return type(v)(v.tensor, v.offset, ap)


def build_phase1():
    nc = bacc.Bacc("TRN2", target_bir_lowering=False, debug=False,
                   num_devices=N_CORES)
    f32, f32r = mybir.dt.float32, mybir.dt.float32r
    u16 = mybir.dt.uint16
    f8 = mybir.dt.float8e4
    imgp = nc.dram_tensor("imgp", [SLABS_PER_CORE, H, W, Z], f32r,
                          kind="ExternalInput")
    bandcw = nc.dram_tensor("bandcw", [128, 128], f32r, kind="ExternalInput")
    s1w = nc.dram_tensor("s1w", [128, 256], f8, kind="ExternalInput")
    # pairs 0..6 ship packed 12-bit (hi u8 + lo i16); the last pair ships
    # raw u16 q12 so the epilogue has no pack chain
    hi_o = nc.dram_tensor("hi", [SLABS_PER_CORE - 2, P_SLAB // 2],
                          mybir.dt.uint8, kind="ExternalOutput")
    lo_o = nc.dram_tensor("lo", [SLABS_PER_CORE - 2, P_SLAB // 2],
                          mybir.dt.int16, kind="ExternalOutput")
    qlast_o = nc.dram_tensor("qlast", [2, P_SLAB], u16,
                             kind="ExternalOutput")

    # w' chunking for PSUM banks: chunks of 8 w' rows (<=496 free each)
    W_CHUNKS = [(i, min(8, HP - i)) for i in range(0, HP, 8)]

    with tile.TileContext(nc) as tc:
        with (
            tc.tile_pool(name="pool", bufs=1) as pool,
            tc.tile_pool(name="pimg", bufs=6) as pimg,
            tc.tile_pool(name="px8", bufs=2) as px8,
            tc.tile_pool(name="pq", bufs=2) as pq,
            tc.tile_pool(name="pv", bufs=4) as pv,
            tc.tile_pool(name="phl", bufs=5) as phl,
            tc.tile_pool(name="psum", bufs=8, space="PSUM") as psum,
        ):
            bandc_t = pool.tile([128, 128], f32r, tag="bandc")
            s1_t = pool.tile([128, 256], f8, tag="s1")
            s1_3 = s1_t[:].rearrange("p (t m) -> p t m", t=2)

            qbia_t = pool.tile([128, 1], mybir.dt.float32, tag="qbia")
            nc.vector.memset(qbia_t[:], QBIA)

            img_tiles = [None] * PAIRS
            x8_tiles = [None] * PAIRS
            q12_tiles = [None] * PAIRS
            v24_tiles = [None] * PAIRS
            hi_tiles = [None] * PAIRS
            lo_tiles = [None] * PAIRS

            def emit_load(p):
                src = imgp[2 * p:2 * p + 2].rearrange("s h w z -> (s h) (w z)")
                t = pimg.tile([128, W * Z], f32r, tag="img")
                img_tiles[p] = t
                nc.sync.dma_start(t[:], src)

            CSPL = 40 * Z  # cast split: w 0:40 on DVE, 40:64 on Pool

            def emit_cast_d(p):
                x8 = px8.tile([128, W * Z], f8, tag="x8")
                x8_tiles[p] = x8
                nc.vector.tensor_scalar(x8[:, 0:CSPL],
                                        img_tiles[p][:, 0:CSPL], 1.0, None,
                                        mybir.AluOpType.mult)

            def emit_cast_p(p):
                nc.gpsimd.tensor_scalar(x8_tiles[p][:, CSPL:],
                                        img_tiles[p][:, CSPL:], 1.0, None,
                                        mybir.AluOpType.mult)

            def emit_pack_v(p):
                # v24 halves on DVE: q12(p) is fully written one pair ago,
                # so these run dependency-free at iteration start
                q3 = q12_tiles[p][:].rearrange("p (f two) -> p two f", two=2)
                va = pv.tile([128, FDH // 2], mybir.dt.float32, tag="va")
                vb = pv.tile([128, FDH // 2], mybir.dt.float32, tag="vb")
                v24_tiles[p] = (va, vb)
                hit = phl.tile([128, FDH], mybir.dt.uint8, tag="hi")
                lot = phl.tile([128, FDH], mybir.dt.int16, tag="lo")
                hi_tiles[p] = hit
                lo_tiles[p] = lot
                for h, v in ((0, va), (1, vb)):
                    sl = slice(h * (FDH // 2), (h + 1) * (FDH // 2))
                    nc.vector.scalar_tensor_tensor(
                        v[:], q3[:, 1, sl], 4096.0, q3[:, 0, sl],
                        mybir.AluOpType.mult, mybir.AluOpType.add)

            def emit_pack_hi(p):
                # hi = round(v24 / 2^16) as u8, both halves on Pool
                for h, v in ((0, v24_tiles[p][0]), (1, v24_tiles[p][1])):
                    sl = slice(h * (FDH // 2), (h + 1) * (FDH // 2))
                    nc.gpsimd.tensor_scalar(hi_tiles[p][:, sl], v[:],
                                            float(2.0 ** -16), None,
                                            mybir.AluOpType.mult)

            def emit_pack_lo(p):
                # lo = v24 - 65536*hi as i16 on DVE (exact)
                for h, v in ((0, v24_tiles[p][0]), (1, v24_tiles[p][1])):
                    sl = slice(h * (FDH // 2), (h + 1) * (FDH // 2))
                    nc.vector.scalar_tensor_tensor(
                        lo_tiles[p][:, sl], hi_tiles[p][:, sl], -65536.0,
                        v[:], mybir.AluOpType.mult, mybir.AluOpType.add)

            def emit_outs(p):
                # 4 writes on the SP queue (loads are long since issued)
                for half in range(2):
                    s = 2 * p + half
                    rows = slice(64 * half + 1, 64 * half + 63)
                    nc.sync.dma_start(
                        hi_o[s].rearrange("(h f) -> h f", h=HP),
                        hi_tiles[p][rows, :])
                    nc.sync.dma_start(
                        lo_o[s].rearrange("(h f) -> h f", h=HP),
                        lo_tiles[p][rows, :])

            # prologue: first data load ahead of the weight DMAs, then the
            # rest of the loads stream back-to-back on the SP queue
            emit_load(0)
            nc.sync.dma_start(bandc_t[:], bandcw[:])
            nc.sync.dma_start(s1_t[:], s1w[:])
            for p in range(1, PAIRS):
                emit_load(p)
            emit_cast_d(0)
            emit_cast_p(0)

            for p in range(PAIRS):
                if p >= 1:
                    emit_pack_v(p - 1)        # DVE: dependency-free packs
                if p + 1 < PAIRS:
                    emit_cast_d(p + 1)
                    emit_cast_p(p + 1)
                if p >= 1:
                    emit_pack_hi(p - 1)       # Pool, after v24 halves

                xv = img_tiles[p][:].rearrange("p (w z) -> p w z", w=W)
                xf3 = x8_tiles[p][:].rearrange("p (w z) -> p w z", w=W)
                q12 = pq.tile([128, FD], u16, tag="q12")
                q12_tiles[p] = q12
                for ci, (w0, wn) in enumerate(W_CHUNKS):
                    ps = psum.tile([128, 8 * HP], f32, tag="ps")
                    out_ap = ps[:, 0:wn * HP]
                    # 4 fp8 DoubleRow matmuls: 8 non-middle tap columns
                    for ti, (dw, dz, st) in enumerate(DR_TAPS):
                        nc.tensor.matmul(
                            out_ap, s1_3, _dr_rhs(xf3, w0, wn, dw, dz, st),
                            start=(ti == 0), stop=False,
                            perf_mode=mybir.MatmulPerfMode.DoubleRow)
                    # middle column (w+1, z+1): band + center diag in f32r
                    nc.tensor.matmul(out_ap, bandc_t[:],
                                     xv[:, w0 + 1:w0 + 1 + wn, 1:1 + HP],
                                     start=False, stop=True)
                    # fused evac + fixed-scale quantize (saturating u16)
                    sl = slice(w0 * HP, (w0 + wn) * HP)
                    nc.scalar.activation(
                        q12[:, sl], out_ap,
                        mybir.ActivationFunctionType.Identity,
                        scale=EVAC_SCALE, bias=qbia_t[:])
                if p >= 1:
                    emit_pack_lo(p - 1)       # DVE, after Pool's hi
                    emit_outs(p - 1)

            # epilogue: last pair ships raw u16 q12 (no pack chain)
            for half in range(2):
                rows = slice(64 * half + 1, 64 * half + 63)
                nc.sync.dma_start(
                    qlast_o[half].rearrange("(h f) -> h f", h=HP),
                    q12_tiles[PAIRS - 1][rows, :])

    nc.finalize()
    return nc


def _stride_runs(rows):
    """Group a sorted int list into (start, stride, count) constant-stride
    runs (each run becomes one strided DMA access pattern)."""
    runs = []
    i, n = 0, len(rows)
    while i < n:
        if i + 1 == n:
            runs.append((rows[i], 1, 1))
            break
        d = rows[i + 1] - rows[i]
        j = i + 1
        while j + 1 < n and rows[j + 1] - rows[j] == d:
            j += 1
        runs.append((rows[i], d, j - i + 1))
        i = j + 1
    return runs


def build_phase2(sel_rows_sorted):
    """sel_rows_sorted: ascending flat row ids (b*C+c); identical program on
    all cores; each core handles one column-chunk of every selected row.
    Consecutive rows are batched into single DMAs."""
    n_sel = len(sel_rows_sorted)
    CHUNK = (H * W * Z) // N_CORES
    nc = bacc.Bacc("TRN2", target_bir_lowering=False, debug=False,
                   num_devices=N_CORES)
    f32 = mybir.dt.float32
    img = nc.dram_tensor("imgchunk", [B * C, CHUNK], f32,
                         kind="ExternalInput")
    out = nc.dram_tensor("sel", [n_sel, CHUNK], f32, kind="ExternalOutput")
    with tile.TileContext(nc):
        j = 0
        engines = [nc.sync, nc.scalar, nc.vector, nc.gpsimd]
        for i, (r0, d, cnt) in enumerate(
                _stride_runs([int(r) for r in sel_rows_sorted])):
            engines[i % len(engines)].dma_start(
                out[j:j + cnt, :], img[r0:r0 + (cnt - 1) * d + 1:d, :])
            j += cnt
    nc.finalize()
    return nc, n_sel


# ---------------------------------------------------------------------------
# host middle
# ---------------------------------------------------------------------------


def unpack_q12(hi, lo, qlast):
    """hi: [n_cores, 14, P_SLAB//2] u8, lo: same-shape i16, qlast:
    [n_cores, 2, P_SLAB] u16 -> q12 [B*C, P_SLAB] int64."""
    n_cores = hi.shape[0]
    v24 = hi.astype(np.int64) * 65536 + lo.astype(np.int64)
    q_odd = v24 >> 12
    q_even = v24 - (q_odd << 12)
    q12 = np.empty((n_cores, SLABS_PER_CORE, HP, FD), np.int64)
    q12[:, :14, :, 0::2] = q_even.reshape(n_cores, 14, HP, FD // 2)
    q12[:, :14, :, 1::2] = q_odd.reshape(n_cores, 14, HP, FD // 2)
    q12[:, 14:] = qlast.astype(np.int64).reshape(n_cores, 2, HP, FD)
    return q12.reshape(B * C, P_SLAB)


def host_middle(img, k, q12, jnp, jax):
    """q12: [B*C, P_SLAB] int64 fixed-grid samples in device (h',w',z')
    order. Returns idx [B, k]."""
    nrows = B * C
    imgf = np.asarray(img)

    def exact_ij(rs, fs):
        hq, rem = np.divmod(fs, HP * HP)
        wq, zq = np.divmod(rem, HP)
        bq, cq = np.divmod(rs, C)
        s = np.zeros(len(rs), np.float32)
        for di in range(3):
            for dj in range(3):
                for dk in range(3):
                    s = s + imgf[bq, cq, hq + di, wq + dj, zq + dk]
        cen = imgf[bq, cq, hq + 1, wq + 1, zq + 1]
        mean_p = (s - cen) / np.float32(26.0)
        return cen * np.float32(100.0) + mean_p

    # saturation / pack-ambiguity repair: q12 near 0 or 4080 means |ij|
    # may have exceeded B_FIX (or hit the hi-rounding tie) on that row ->
    # recompute the whole row exactly
    sat_rows = np.unique(np.nonzero((q12 <= 1) | (q12 >= 4079))[0])
    ij_dec = (q12.astype(np.float64) - QBIA) * ULP
    for r in sat_rows:
        fs = np.arange(P_SLAB)
        ij_dec[r] = exact_ij(np.full(P_SLAB, r), fs).astype(np.float64)

    # exact global min/max: candidates = decoded values near the decoded
    # extremes (true extreme is within decode ulp + device-arith error
    # of the decoded one; 0.1 ij-units covers the arithmetic tail)
    mn_d = ij_dec.min()
    mx_d = ij_dec.max()
    win = 2.5 * ULP + 0.5
    cand = (ij_dec <= mn_d + win) | (ij_dec >= mx_d - win)
    crs, cfs = np.nonzero(cand)
    cij = exact_ij(crs, cfs)
    mn = np.float32(cij.min())
    mx = np.float32(cij.max())

    # provisional bins + boundary flags from decoded values
    qc = (ij_dec - mn) * (np.float64(BINS) / np.float64(mx - mn))
    binf = np.floor(qc)
    frac = qc - binf
    bins = np.clip(binf, 0, BINS - 1).astype(np.int64)
    flag = (frac < FLAG_T) | (frac > 1.0 - FLAG_T) | (binf < 0) | \
           (binf > BINS - 1)
    del qc, binf, frac, ij_dec

    hist = np.zeros((nrows, BINS), np.int64)
    for r in range(nrows):
        hist[r] = np.bincount(bins[r], minlength=BINS)

    # flagged: recompute exactly in reference f32 arithmetic and move count
    rs, fs = np.nonzero(flag)
    ij_ref = exact_ij(rs, fs)
    q = (ij_ref - mn) / (mx - mn)
    true_bin = np.clip(np.floor(q * np.float32(BINS)), 0, BINS - 1).astype(np.int64)
    dev_bin = bins[rs, fs]
    np.subtract.at(hist, (rs, dev_bin), 1)
    np.add.at(hist, (rs, true_bin), 1)

    # entropy + topk exactly as reference (jax CPU)
    cpu = jax.devices("cpu")[0]
    with jax.default_device(cpu):
        h = jnp.asarray(hist.astype(np.float32))
        p = h / DENOM
        h_tem = -p * jnp.log(jnp.clip(p, 1e-40)) / np.float32(np.log(2.0))
        ent = h_tem.sum(axis=1).reshape(B, C)
        _, idx = jax.lax.top_k(ent, int(k))
        idx = np.asarray(idx)
    return idx, hist, (mn, mx)


LAST_NCS = [None, None]  # (nc1, nc2) from the most recent run_full


def run_full(img, k, trace=False):
    import jax
    import jax.numpy as jnp
    img = np.asarray(img, dtype=np.float32)
    k = int(k)

    nc1 = build_phase1()
    bandc = build_bandc()
    s1 = build_band8()
    imgr = img.reshape(B * C, H, W, Z)
    in_maps = [{"imgp": np.ascontiguousarray(imgr[16 * c:16 * c + 16]),
                "bandcw": bandc, "s1w": s1}
               for c in range(N_CORES)]
    res1 = run_bass_kernel_spmd(nc1, in_maps, core_ids=list(range(N_CORES)),
                                trace=trace)
    hi = np.stack([res1.results[c]["hi"] for c in range(N_CORES)], 0)
    lo = np.stack([res1.results[c]["lo"] for c in range(N_CORES)], 0)
    qlast = np.stack([res1.results[c]["qlast"] for c in range(N_CORES)], 0)
    q12 = unpack_q12(hi, lo, qlast)

    idx, hist, mnmx = host_middle(img, k, q12, jnp, jax)

    # phase 2: device gather of selected slabs, column-sharded over cores;
    # device writes sorted row order, host restores top-k order
    rows_flat = np.array([int(b * C + ch) for b in range(B) for ch in idx[b]])
    order = np.argsort(rows_flat, kind="stable")
    rows_sorted = rows_flat[order]
    inv = np.empty_like(order)
    inv[order] = np.arange(len(order))

    nc2, n_sel = build_phase2(rows_sorted.tolist())
    LAST_NCS[0], LAST_NCS[1] = nc1, nc2
    CHUNK = (H * W * Z) // N_CORES
    img2 = img.reshape(B * C, H * W * Z)
    in2 = [{"imgchunk": np.ascontiguousarray(img2[:, c * CHUNK:(c + 1) * CHUNK])}
           for c in range(N_CORES)]
    res2 = run_bass_kernel_spmd(nc2, in2, core_ids=list(range(N_CORES)),
                                trace=trace)

    out_sorted = np.zeros((n_sel, H * W * Z), np.float32)
    for c in range(N_CORES):
        out_sorted[:, c * CHUNK:(c + 1) * CHUNK] = res2.results[c]["sel"]
    out = out_sorted[inv].reshape(B, k, H, W, Z)
    return out, (res1, res2)


def kernel(**inputs):
    """Entry point: full inputs in, full output out."""
    img = np.asarray(inputs["img"], dtype=np.float32)
    k = int(np.asarray(inputs["k"]))
    out, _ = run_full(img, k)
    return out.astype(np.float32)


# revision 39
# speedup vs baseline: 1.0991x; 1.0331x over previous
"""Device kernels + host middle for nn_Entropy_Hist (3x3x3 window entropy
histogram + top-k channel gather) on 8 trn2 cores.

Phase 1 (device): per core 16 channel slabs (8 pairs, partition = h of 2
slabs). Per pair: ONE contiguous DMA load [128, 64*64], a single fp8
cast pass of x on DVE, then per w-chunk FIVE matmuls: four fp8
DoubleRow band matmuls whose k-tile pairs cover the 8 non-middle (w,z)
tap columns (each tap column is a 3-h band sum), plus one f32r bandc
matmul for the middle column (w+1,z+1) carrying its 2 h-neighbors at
f32 precision and the center term on the diagonal. Then a FUSED
evac+quantize: one activation op PSUM -> u16 with a FIXED compile-time
scale (no per-pair absmax / reciprocal on the critical path):
    q16 = round_half_even( ij * (32767/B_FIX) + 32767 ),  saturating.
B_FIX bounds |ij|; saturation shows up as q16 in {0, 65535} which the
host detects and repairs by exact recompute of the affected rows.

Host middle: decode ij from q16 with the fixed scale, locate exact
global min/max among decoded-extreme candidates (recomputed exactly),
bin all samples, recompute near-boundary (flagged) samples exactly,
entropy + top-k as reference.

Phase 2 (device): gather selected channel rows, column-sharded across
cores, consecutive selected rows batched into single DMAs (device emits
sorted row order; host restores top-k order).
"""

import ml_dtypes
import numpy as np

import concourse.bass as bass
import concourse.bacc as bacc
import concourse.mybir as mybir
import concourse.tile as tile
from concourse.bass_utils import run_bass_kernel_spmd

N_CORES = 8
B, C, H, W, Z = 2, 64, 64, 64, 64
HP = H - 2              # 62 valid per spatial dim
FD = HP * HP            # 3844 free elems per partition (w', z')
P_SLAB = HP * HP * HP   # 238328 voxels per slab
SLABS_PER_CORE = (B * C) // N_CORES  # 16
PAIRS = SLABS_PER_CORE // 2          # 8
K26 = np.float32(1.0) / np.float32(26.0)
CDIAG = np.float32(100.0) - K26      # center coefficient
BINS = 256
DENOM = (H + 2) * (W + 2) * (Z + 2)

# fixed 12-bit quantization grid: q12 = round(ij * QS + QBIA) in [0, 4080].
# Each adjacent sample pair is packed as v24 = q_even + 4096*q_odd and
# emitted as u8 hi = round(v24/2^16) (exact: ties impossible for q>=2)
# plus i16 lo = v24 - hi*2^16  -> 1.5 bytes/sample.
B_FIX = 640.0            # bound on |ij| (dataset max ~542); q12 near 0 or
QBIA = 2040.0            # 4080 is detected on host and repaired exactly
QS = 2040.0 / B_FIX
ULP = B_FIX / 2040.0     # decode step (7.5% of a bin)
FLAG_T = 0.088           # bin-fraction margin -> host recomputes exactly
                         # (0.5 ulp = 3.8% + device arith 3.6% + safety)
FDH = FD // 2            # 1922 packed sample-pairs per partition

# fp8 weight grid: the X2 (non-center) taps run through fp8 DoubleRow
# matmuls with weight BETA; the evac rescales by K26/BETA
BETA = np.float32(0.0390625)          # 1/25.6, exact in e4m3
SCORR = np.float64(K26) / np.float64(BETA)          # evac scale
CDIAG_ADJ = np.float32(np.float64(CDIAG) / SCORR)   # center diag pre-descale
EVAC_SCALE = float(np.float64(SCORR) * np.float64(QS))  # psum -> q units


def build_band(w=BETA):
    """[128,128] f32: col m sums rows m-1..m+1 (within each 64 block) with
    weight w. Cols 0,63,64,127 are all-zero, so the garbage partitions
    hold exact 0 (which quantizes to QBIA: harmless, never read)."""
    band = np.zeros((128, 128), np.float32)
    for blk in (0, 64):
        for m in range(1, 63):
            for k in (m - 1, m, m + 1):
                band[blk + k, blk + m] = w
    return band


def build_bandc():
    """beta-band + CDIAG_ADJ * I on valid cols: the middle tap column
    (w+1, z+1) in f32r — its two h-neighbors at weight BETA plus the
    center term on the diagonal (evac scale SCORR restores CDIAG)."""
    band = build_band()
    for blk in (0, 64):
        for m in range(1, 63):
            band[blk + m, blk + m] += CDIAG_ADJ
    return band


def build_band8():
    """fp8 DoubleRow stationary [128, 2*128] (k-tile major):
    S1 = [band | band] — both k-tiles contract a tap column with the
    3-h band at weight BETA."""
    b8 = build_band().astype(ml_dtypes.float8_e4m3)
    s1 = np.concatenate([b8, b8], axis=1)
    return s1


# the 8 non-middle (w,z) tap columns, paired into 4 DoubleRow matmuls:
# each entry is (dw, dz, ktile_stride_elems) with the second k-tile at
# flat offset +stride in the [p, w, z] fp8 tile (z stride 1, w stride Z)
DR_TAPS = [
    (0, 0, 1),       # (0,0) + (0,1)
    (0, 2, Z - 2),   # (0,2) + (1,0)
    (1, 2, Z - 2),   # (1,2) + (2,0)
    (2, 1, 1),       # (2,1) + (2,2)
]


def _dr_rhs(xf3, w0, wn, dw, dz, stride):
    """Moving AP [128, 2(k-tile), wn, 62]: k-tile 0 at (w0+dw, dz), k-tile
    1 at flat offset +stride (overlapping strided dims)."""
    v = xf3[:, w0 + dw:w0 + dw + wn, dz:dz + HP]
    ap = [list(v.ap[0]), [stride, 2], list(v.ap[1]), list(v.ap[2])]
    return type(v)(v.tensor, v.offset, ap)


def build_phase1():
    nc = bacc.Bacc("TRN2", target_bir_lowering=False, debug=False,
                   num_devices=N_CORES)
    f32, f32r = mybir.dt.float32, mybir.dt.float32r
    u16 = mybir.dt.uint16
    f8 = mybir.dt.float8e4
    imgp = nc.dram_tensor("imgp", [SLABS_PER_CORE, H, W, Z], f32r,
                          kind="ExternalInput")
    bandcw = nc.dram_tensor("bandcw", [128, 128], f32r, kind="ExternalInput")
    s1w = nc.dram_tensor("s1w", [128, 256], f8, kind="ExternalInput")
    # pairs 0..5 ship packed 12-bit (hi u8 + lo i16); the last two pairs
    # ship raw u16 q12 so the pipeline tail has no pack chain
    hi_o = nc.dram_tensor("hi", [SLABS_PER_CORE - 4, P_SLAB // 2],
                          mybir.dt.uint8, kind="ExternalOutput")
    lo_o = nc.dram_tensor("lo", [SLABS_PER_CORE - 4, P_SLAB // 2],
                          mybir.dt.int16, kind="ExternalOutput")
    qlast_o = nc.dram_tensor("qlast", [4, P_SLAB], u16,
                             kind="ExternalOutput")

    # w' chunking for PSUM banks: chunks of 8 w' rows (<=496 free each)
    W_CHUNKS = [(i, min(8, HP - i)) for i in range(0, HP, 8)]

    with tile.TileContext(nc) as tc:
        with (
            tc.tile_pool(name="pool", bufs=1) as pool,
            tc.tile_pool(name="pimg", bufs=6) as pimg,
            tc.tile_pool(name="px8", bufs=2) as px8,
            tc.tile_pool(name="pq", bufs=2) as pq,
            tc.tile_pool(name="pv", bufs=4) as pv,
            tc.tile_pool(name="phl", bufs=5) as phl,
            tc.tile_pool(name="psum", bufs=8, space="PSUM") as psum,
        ):
            bandc_t = pool.tile([128, 128], f32r, tag="bandc")
            s1_t = pool.tile([128, 256], f8, tag="s1")
            s1_3 = s1_t[:].rearrange("p (t m) -> p t m", t=2)

            qbia_t = pool.tile([128, 1], mybir.dt.float32, tag="qbia")
            nc.vector.memset(qbia_t[:], QBIA)

            img_tiles = [None] * PAIRS
            x8_tiles = [None] * PAIRS
            q12_tiles = [None] * PAIRS
            v24_tiles = [None] * PAIRS
            hi_tiles = [None] * PAIRS
            lo_tiles = [None] * PAIRS

            def emit_load(p):
                src = imgp[2 * p:2 * p + 2].rearrange("s h w z -> (s h) (w z)")
                t = pimg.tile([128, W * Z], f32r, tag="img")
                img_tiles[p] = t
                nc.sync.dma_start(t[:], src)

            CSPL = 40 * Z  # cast split: w 0:40 on DVE, 40:64 on Pool

            def emit_cast_d(p):
                x8 = px8.tile([128, W * Z], f8, tag="x8")
                x8_tiles[p] = x8
                nc.vector.tensor_scalar(x8[:, 0:CSPL],
                                        img_tiles[p][:, 0:CSPL], 1.0, None,
                                        mybir.AluOpType.mult)

            def emit_cast_p(p):
                nc.gpsimd.tensor_scalar(x8_tiles[p][:, CSPL:],
                                        img_tiles[p][:, CSPL:], 1.0, None,
                                        mybir.AluOpType.mult)

            def emit_pack_v(p):
                # v24 halves on DVE: q12(p) is fully written one pair ago,
                # so these run dependency-free at iteration start
                q3 = q12_tiles[p][:].rearrange("p (f two) -> p two f", two=2)
                va = pv.tile([128, FDH // 2], mybir.dt.float32, tag="va")
                vb = pv.tile([128, FDH // 2], mybir.dt.float32, tag="vb")
                v24_tiles[p] = (va, vb)
                hit = phl.tile([128, FDH], mybir.dt.uint8, tag="hi")
                lot = phl.tile([128, FDH], mybir.dt.int16, tag="lo")
                hi_tiles[p] = hit
                lo_tiles[p] = lot
                for h, v in ((0, va), (1, vb)):
                    sl = slice(h * (FDH // 2), (h + 1) * (FDH // 2))
                    nc.vector.scalar_tensor_tensor(
                        v[:], q3[:, 1, sl], 4096.0, q3[:, 0, sl],
                        mybir.AluOpType.mult, mybir.AluOpType.add)

            def emit_pack_hi(p):
                # hi = round(v24 / 2^16) as u8, both halves on Pool
                for h, v in ((0, v24_tiles[p][0]), (1, v24_tiles[p][1])):
                    sl = slice(h * (FDH // 2), (h + 1) * (FDH // 2))
                    nc.gpsimd.tensor_scalar(hi_tiles[p][:, sl], v[:],
                                            float(2.0 ** -16), None,
                                            mybir.AluOpType.mult)

            def emit_pack_lo(p):
                # lo = v24 - 65536*hi as i16 on DVE (exact)
                for h, v in ((0, v24_tiles[p][0]), (1, v24_tiles[p][1])):
                    sl = slice(h * (FDH // 2), (h + 1) * (FDH // 2))
                    nc.vector.scalar_tensor_tensor(
                        lo_tiles[p][:, sl], hi_tiles[p][:, sl], -65536.0,
                        v[:], mybir.AluOpType.mult, mybir.AluOpType.add)

            def emit_outs(p):
                # 4 writes on the SP queue (loads are long since issued)
                for half in range(2):
                    s = 2 * p + half
                    rows = slice(64 * half + 1, 64 * half + 63)
                    nc.sync.dma_start(
                        hi_o[s].rearrange("(h f) -> h f", h=HP),
                        hi_tiles[p][rows, :])
                    nc.sync.dma_start(
                        lo_o[s].rearrange("(h f) -> h f", h=HP),
                        lo_tiles[p][rows, :])

            # prologue: first data load ahead of the weight DMAs, then the
            # rest of the loads stream back-to-back on the SP queue
            emit_load(0)
            nc.sync.dma_start(bandc_t[:], bandcw[:])
            nc.sync.dma_start(s1_t[:], s1w[:])
            for p in range(1, PAIRS):
                emit_load(p)
            emit_cast_d(0)
            emit_cast_p(0)

            RAW_PAIRS = (PAIRS - 2, PAIRS - 1)

            for p in range(PAIRS):
                if 1 <= p and p - 1 not in RAW_PAIRS:
                    emit_pack_v(p - 1)        # DVE: dependency-free packs
                if p + 1 < PAIRS:
                    emit_cast_d(p + 1)
                    emit_cast_p(p + 1)
                if 1 <= p and p - 1 not in RAW_PAIRS:
                    emit_pack_hi(p - 1)       # Pool, after v24 halves

                xv = img_tiles[p][:].rearrange("p (w z) -> p w z", w=W)
                xf3 = x8_tiles[p][:].rearrange("p (w z) -> p w z", w=W)
                q12 = pq.tile([128, FD], u16, tag="q12")
                q12_tiles[p] = q12
                for ci, (w0, wn) in enumerate(W_CHUNKS):
                    ps = psum.tile([128, 8 * HP], f32, tag="ps")
                    out_ap = ps[:, 0:wn * HP]
                    # 4 fp8 DoubleRow matmuls: 8 non-middle tap columns
                    for ti, (dw, dz, st) in enumerate(DR_TAPS):
                        nc.tensor.matmul(
                            out_ap, s1_3, _dr_rhs(xf3, w0, wn, dw, dz, st),
                            start=(ti == 0), stop=False,
                            perf_mode=mybir.MatmulPerfMode.DoubleRow)
                    # middle column (w+1, z+1): band + center diag in f32r
                    nc.tensor.matmul(out_ap, bandc_t[:],
                                     xv[:, w0 + 1:w0 + 1 + wn, 1:1 + HP],
                                     start=False, stop=True)
                    # fused evac + fixed-scale quantize (saturating u16)
                    sl = slice(w0 * HP, (w0 + wn) * HP)
                    nc.scalar.activation(
                        q12[:, sl], out_ap,
                        mybir.ActivationFunctionType.Identity,
                        scale=EVAC_SCALE, bias=qbia_t[:])
                if 1 <= p and p - 1 not in RAW_PAIRS:
                    emit_pack_lo(p - 1)       # DVE, after Pool's hi
                    emit_outs(p - 1)
                if p in RAW_PAIRS:
                    # raw u16 writes right behind this pair's evacs
                    for half in range(2):
                        rows = slice(64 * half + 1, 64 * half + 63)
                        nc.sync.dma_start(
                            qlast_o[2 * (p - RAW_PAIRS[0]) + half]
                            .rearrange("(h f) -> h f", h=HP),
                            q12_tiles[p][rows, :])

    nc.finalize()
    return nc


def _stride_runs(rows):
    """Group a sorted int list into (start, stride, count) constant-stride
    runs (each run becomes one strided DMA access pattern)."""
    runs = []
    i, n = 0, len(rows)
    while i < n:
        if i + 1 == n:
            runs.append((rows[i], 1, 1))
            break
        d = rows[i + 1] - rows[i]
        j = i + 1
        while j + 1 < n and rows[j + 1] - rows[j] == d:
            j += 1
        runs.append((rows[i], d, j - i + 1))
        i = j + 1
    return runs


def build_phase2(sel_rows_sorted):
    """sel_rows_sorted: ascending flat row ids (b*C+c); identical program on
    all cores; each core handles one column-chunk of every selected row.
    Consecutive rows are batched into single DMAs."""
    n_sel = len(sel_rows_sorted)
    CHUNK = (H * W * Z) // N_CORES
    nc = bacc.Bacc("TRN2", target_bir_lowering=False, debug=False,
                   num_devices=N_CORES)
    f32 = mybir.dt.float32
    img = nc.dram_tensor("imgchunk", [B * C, CHUNK], f32,
                         kind="ExternalInput")
    out = nc.dram_tensor("sel", [n_sel, CHUNK], f32, kind="ExternalOutput")
    with tile.TileContext(nc):
        j = 0
        engines = [nc.sync, nc.scalar, nc.vector, nc.gpsimd]
        for i, (r0, d, cnt) in enumerate(
                _stride_runs([int(r) for r in sel_rows_sorted])):
            engines[i % len(engines)].dma_start(
                out[j:j + cnt, :], img[r0:r0 + (cnt - 1) * d + 1:d, :])
            j += cnt
    nc.finalize()
    return nc, n_sel


# ---------------------------------------------------------------------------
# host middle
# ---------------------------------------------------------------------------


def unpack_q12(hi, lo, qlast):
    """hi: [n_cores, 12, P_SLAB//2] u8, lo: same-shape i16, qlast:
    [n_cores, 4, P_SLAB] u16 -> q12 [B*C, P_SLAB] int64."""
    n_cores = hi.shape[0]
    v24 = hi.astype(np.int64) * 65536 + lo.astype(np.int64)
    q_odd = v24 >> 12
    q_even = v24 - (q_odd << 12)
    q12 = np.empty((n_cores, SLABS_PER_CORE, HP, FD), np.int64)
    q12[:, :12, :, 0::2] = q_even.reshape(n_cores, 12, HP, FD // 2)
    q12[:, :12, :, 1::2] = q_odd.reshape(n_cores, 12, HP, FD // 2)
    q12[:, 12:] = qlast.astype(np.int64).reshape(n_cores, 4, HP, FD)
    return q12.reshape(B * C, P_SLAB)


def host_middle(img, k, q12, jnp, jax):
    """q12: [B*C, P_SLAB] int64 fixed-grid samples in device (h',w',z')
    order. Returns idx [B, k]."""
    nrows = B * C
    imgf = np.asarray(img)

    def exact_ij(rs, fs):
        hq, rem = np.divmod(fs, HP * HP)
        wq, zq = np.divmod(rem, HP)
        bq, cq = np.divmod(rs, C)
        s = np.zeros(len(rs), np.float32)
        for di in range(3):
            for dj in range(3):
                for dk in range(3):
                    s = s + imgf[bq, cq, hq + di, wq + dj, zq + dk]
        cen = imgf[bq, cq, hq + 1, wq + 1, zq + 1]
        mean_p = (s - cen) / np.float32(26.0)
        return cen * np.float32(100.0) + mean_p

    # saturation / pack-ambiguity repair: q12 near 0 or 4080 means |ij|
    # may have exceeded B_FIX (or hit the hi-rounding tie) on that row ->
    # recompute the whole row exactly
    sat_rows = np.unique(np.nonzero((q12 <= 1) | (q12 >= 4079))[0])
    ij_dec = (q12.astype(np.float64) - QBIA) * ULP
    for r in sat_rows:
        fs = np.arange(P_SLAB)
        ij_dec[r] = exact_ij(np.full(P_SLAB, r), fs).astype(np.float64)

    # exact global min/max: candidates = decoded values near the decoded
    # extremes (true extreme is within decode ulp + device-arith error
    # of the decoded one; 0.1 ij-units covers the arithmetic tail)
    mn_d = ij_dec.min()
    mx_d = ij_dec.max()
    win = 2.5 * ULP + 0.5
    cand = (ij_dec <= mn_d + win) | (ij_dec >= mx_d - win)
    crs, cfs = np.nonzero(cand)
    cij = exact_ij(crs, cfs)
    mn = np.float32(cij.min())
    mx = np.float32(cij.max())

    # provisional bins + boundary flags from decoded values
    qc = (ij_dec - mn) * (np.float64(BINS) / np.float64(mx - mn))
    binf = np.floor(qc)
    frac = qc - binf
    bins = np.clip(binf, 0, BINS - 1).astype(np.int64)
    flag = (frac < FLAG_T) | (frac > 1.0 - FLAG_T) | (binf < 0) | \
           (binf > BINS - 1)
    del qc, binf, frac, ij_dec

    hist = np.zeros((nrows, BINS), np.int64)
    for r in range(nrows):
        hist[r] = np.bincount(bins[r], minlength=BINS)

    # flagged: recompute exactly in reference f32 arithmetic and move count
    rs, fs = np.nonzero(flag)
    ij_ref = exact_ij(rs, fs)
    q = (ij_ref - mn) / (mx - mn)
    true_bin = np.clip(np.floor(q * np.float32(BINS)), 0, BINS - 1).astype(np.int64)
    dev_bin = bins[rs, fs]
    np.subtract.at(hist, (rs, dev_bin), 1)
    np.add.at(hist, (rs, true_bin), 1)

    # entropy + topk exactly as reference (jax CPU)
    cpu = jax.devices("cpu")[0]
    with jax.default_device(cpu):
        h = jnp.asarray(hist.astype(np.float32))
        p = h / DENOM
        h_tem = -p * jnp.log(jnp.clip(p, 1e-40)) / np.float32(np.log(2.0))
        ent = h_tem.sum(axis=1).reshape(B, C)
        _, idx = jax.lax.top_k(ent, int(k))
        idx = np.asarray(idx)
    return idx, hist, (mn, mx)


LAST_NCS = [None, None]  # (nc1, nc2) from the most recent run_full


def run_full(img, k, trace=False):
    import jax
    import jax.numpy as jnp
    img = np.asarray(img, dtype=np.float32)
    k = int(k)

    nc1 = build_phase1()
    bandc = build_bandc()
    s1 = build_band8()
    imgr = img.reshape(B * C, H, W, Z)
    in_maps = [{"imgp": np.ascontiguousarray(imgr[16 * c:16 * c + 16]),
                "bandcw": bandc, "s1w": s1}
               for c in range(N_CORES)]
    res1 = run_bass_kernel_spmd(nc1, in_maps, core_ids=list(range(N_CORES)),
                                trace=trace)
    hi = np.stack([res1.results[c]["hi"] for c in range(N_CORES)], 0)
    lo = np.stack([res1.results[c]["lo"] for c in range(N_CORES)], 0)
    qlast = np.stack([res1.results[c]["qlast"] for c in range(N_CORES)], 0)
    q12 = unpack_q12(hi, lo, qlast)

    idx, hist, mnmx = host_middle(img, k, q12, jnp, jax)

    # phase 2: device gather of selected slabs, column-sharded over cores;
    # device writes sorted row order, host restores top-k order
    rows_flat = np.array([int(b * C + ch) for b in range(B) for ch in idx[b]])
    order = np.argsort(rows_flat, kind="stable")
    rows_sorted = rows_flat[order]
    inv = np.empty_like(order)
    inv[order] = np.arange(len(order))

    nc2, n_sel = build_phase2(rows_sorted.tolist())
    LAST_NCS[0], LAST_NCS[1] = nc1, nc2
    CHUNK = (H * W * Z) // N_CORES
    img2 = img.reshape(B * C, H * W * Z)
    in2 = [{"imgchunk": np.ascontiguousarray(img2[:, c * CHUNK:(c + 1) * CHUNK])}
           for c in range(N_CORES)]
    res2 = run_bass_kernel_spmd(nc2, in2, core_ids=list(range(N_CORES)),
                                trace=trace)

    out_sorted = np.zeros((n_sel, H * W * Z), np.float32)
    for c in range(N_CORES):
        out_sorted[:, c * CHUNK:(c + 1) * CHUNK] = res2.results[c]["sel"]
    out = out_sorted[inv].reshape(B, k, H, W, Z)
    return out, (res1, res2)


def kernel(**inputs):
    """Entry point: full inputs in, full output out."""
    img = np.asarray(inputs["img"], dtype=np.float32)
    k = int(np.asarray(inputs["k"]))
    out, _ = run_full(img, k)
    return out.astype(np.float32)
